# revision 2
# baseline (speedup 1.0000x reference)
"""MoE gating kernel for Trainium2 (Bass/Tile), 8-core data parallel.

Problem: x:[4,4096,2048] f32, W:[8,2048] f32, b:[8] f32
  logits = x @ W.T + b            -> [B,S,8]
  top2 over experts               -> values, indices
  sparse = scatter(top2, -inf)    -> [B,S,8]
  returns (sparse_logits [4,4096,8] f32, indices [4,4096,2] i32,
           gate_logit [16384,8] f32)

Sharding: tokens (B*S = 16384) split evenly across 8 cores; W/b replicated.

Per-core kernel (2048 tokens):
  for each 128-token tile:
    DMA x tile [128, 2048] (natural layout, contiguous)
    PE-transpose 16x [128,128] blocks -> PSUM -> copy to SBUF (xT)
    16 accumulating matmuls: lhsT = xT block [h,t], rhs = Wt chunk [h,8]
      -> PSUM logits [128 tokens, 8 experts]
    DVE: +b, max (sorts 8 desc), max_index, mask = logits >= 2nd max,
      sparse = where(mask, logits, -inf)
    DMA out gate/sparse/idx tiles
"""

import sys

for _p in ("/opt/trn_rl_repo", "/root/.axon_site"):
    if _p not in sys.path:
        sys.path.insert(0, _p)

from contextlib import ExitStack

import numpy as np

import concourse.bacc as bacc
import concourse.bass as bass
import concourse.mybir as mybir
import concourse.tile as tile
from concourse.bass_utils import run_bass_kernel_spmd
from concourse.masks import make_identity

B, S, H, E, TOPK = 4, 4096, 2048, 8, 2
N_CORES = 8
TOKENS = B * S
TOK_PER_CORE = TOKENS // N_CORES  # 2048
P = 128                           # partition tile (tokens per tile)
N_TILES = TOK_PER_CORE // P       # 16
HC = H // P                       # 16 h-chunks
F32 = mybir.dt.float32
NEG_INF = float("-inf")

_cache = {}


def _build():
    nc = bacc.Bacc("TRN2", target_bir_lowering=False, debug=False)

    x_d = nc.dram_tensor("x", [TOK_PER_CORE, H], F32, kind="ExternalInput")
    wt_d = nc.dram_tensor("wt", [P, HC * E], F32, kind="ExternalInput")
    bb_d = nc.dram_tensor("bb", [P, E], F32, kind="ExternalInput")
    gate_d = nc.dram_tensor("gate", [TOK_PER_CORE, E], F32, kind="ExternalOutput")
    sp_d = nc.dram_tensor("sparse", [TOK_PER_CORE, E], F32, kind="ExternalOutput")
    idx_d = nc.dram_tensor("idx", [TOK_PER_CORE, TOPK], mybir.dt.int32,
                           kind="ExternalOutput")

    with tile.TileContext(nc) as tc:
        with ExitStack() as ctx:
            consts = ctx.enter_context(tc.tile_pool(name="consts", bufs=1))
            xpool = ctx.enter_context(tc.tile_pool(name="xin", bufs=3))
            xtpool = ctx.enter_context(tc.tile_pool(name="xT", bufs=2))
            trps = ctx.enter_context(
                tc.tile_pool(name="trps", bufs=4, space="PSUM"))
            lgps = ctx.enter_context(
                tc.tile_pool(name="lgps", bufs=2, space="PSUM"))
            spool = ctx.enter_context(tc.tile_pool(name="small", bufs=4))

            wt_sb = consts.tile([P, HC * E], F32)
            nc.sync.dma_start(wt_sb[:], wt_d[:, :])
            bb_sb = consts.tile([P, E], F32)
            nc.sync.dma_start(bb_sb[:], bb_d[:, :])
            ident = consts.tile([P, P], F32)
            make_identity(nc, ident[:])

            for i in range(N_TILES):
                xt = xpool.tile([P, H], F32)
                nc.sync.dma_start(xt[:], x_d[i * P:(i + 1) * P, :])

                xT = xtpool.tile([P, H], F32)
                # 4 transposes per PSUM bank tile, then one wide copy out
                for g in range(4):
                    ps = trps.tile([P, 512], F32, tag="trp")
                    for j in range(4):
                        c = g * 4 + j
                        nc.tensor.transpose(
                            ps[:, j * P:(j + 1) * P],
                            xt[:, c * P:(c + 1) * P], ident[:])
                    if g % 2 == 0:
                        nc.vector.tensor_copy(
                            xT[:, g * 512:(g + 1) * 512], ps[:])
                    else:
                        nc.scalar.copy(xT[:, g * 512:(g + 1) * 512], ps[:])

                lg = lgps.tile([P, E], F32, tag="lg")
                for c in range(HC):
                    nc.tensor.matmul(
                        lg[:],
                        xT[:, c * P:(c + 1) * P],
                        wt_sb[:, c * E:(c + 1) * E],
                        start=(c == 0),
                        stop=(c == HC - 1),
                    )

                gate = spool.tile([P, E], F32, tag="gate")
                nc.vector.tensor_add(gate[:], lg[:], bb_sb[:])

                mx8 = spool.tile([P, 8], F32, tag="mx8")
                nc.vector.max(out=mx8[:], in_=gate[:])
                ix8 = spool.tile([P, 8], mybir.dt.uint32, tag="ix8")
                nc.vector.max_index(out=ix8[:], in_max=mx8[:], in_values=gate[:])
                ix2 = spool.tile([P, TOPK], mybir.dt.int32, tag="ix2")
                nc.vector.tensor_copy(ix2[:], ix8[:, 0:TOPK])

                keep = spool.tile([P, E], mybir.dt.uint32, tag="keep")
                nc.vector.tensor_scalar(
                    out=keep[:], in0=gate[:], scalar1=mx8[:, 1:2], scalar2=None,
                    op0=mybir.AluOpType.is_ge)
                sp = spool.tile([P, E], F32, tag="sp")
                nc.vector.memset(sp[:], NEG_INF)
                nc.vector.copy_predicated(sp[:], keep[:], gate[:])

                nc.scalar.dma_start(gate_d[i * P:(i + 1) * P, :], gate[:])
                nc.scalar.dma_start(sp_d[i * P:(i + 1) * P, :], sp[:])
                nc.scalar.dma_start(idx_d[i * P:(i + 1) * P, :], ix2[:])

    nc.compile()
    return nc


def _get_nc():
    if "nc" not in _cache:
        _cache["nc"] = _build()
    return _cache["nc"]


def kernel(x: np.ndarray, W: np.ndarray, b: np.ndarray):
    x = np.ascontiguousarray(np.asarray(x, dtype=np.float32)).reshape(TOKENS, H)
    W = np.asarray(W, dtype=np.float32)
    b = np.asarray(b, dtype=np.float32)

    # Wt packed: [p, c*8+e] = W[e, c*128+p]
    wt = np.ascontiguousarray(
        W.T.reshape(HC, P, E).transpose(1, 0, 2).reshape(P, HC * E))
    bb = np.ascontiguousarray(np.broadcast_to(b, (P, E)))

    in_maps = [
        {"x": x[i * TOK_PER_CORE:(i + 1) * TOK_PER_CORE], "wt": wt, "bb": bb}
        for i in range(N_CORES)
    ]
    nc = _get_nc()
    res = run_bass_kernel_spmd(nc, in_maps, list(range(N_CORES)))

    gate = np.concatenate([r["gate"] for r in res.results], axis=0)
    sparse = np.concatenate([r["sparse"] for r in res.results], axis=0)
    idx = np.concatenate([r["idx"] for r in res.results], axis=0)

    return (
        sparse.reshape(B, S, E),
        idx.reshape(B, S, TOPK).astype(np.int32),
        gate.reshape(TOKENS, E),
    )


if __name__ == "__main__":
    rng = np.random.default_rng(0)
    x = rng.standard_normal((B, S, H), dtype=np.float32)
    W = (rng.standard_normal((E, H), dtype=np.float32) / np.sqrt(H)).astype(
        np.float32)
    b = np.zeros((E,), dtype=np.float32)
    sp, ix, gl = kernel(x=x, W=W, b=b)
    print("shapes:", sp.shape, ix.shape, gl.shape, sp.dtype, ix.dtype, gl.dtype)


# revision 6
# speedup vs baseline: 1.1222x; 1.1222x over previous
"""MoE gating kernel for Trainium2 (Bass/Tile), 8-core data parallel.

Problem: x:[4,4096,2048] f32, W:[8,2048] f32, b:[8] f32
  logits = x @ W.T + b            -> [B,S,8]
  top2 over experts               -> values, indices
  sparse = scatter(top2, -inf)    -> [B,S,8]
  returns (sparse_logits [4,4096,8] f32, indices [4,4096,2] i32,
           gate_logit [16384,8] f32)

Sharding: tokens (B*S = 16384) split evenly across 8 cores; W/b replicated.

Per-core kernel (2048 tokens):
  for each 128-token tile:
    DMA x tile [128, 2048] (natural layout, contiguous)
    PE-transpose 16x [128,128] blocks -> PSUM -> copy to SBUF (xT)
    16 accumulating matmuls: lhsT = xT block [h,t], rhs = Wt chunk [h,8]
      -> PSUM logits [128 tokens, 8 experts]
    DVE: +b, max (sorts 8 desc), max_index, mask = logits >= 2nd max,
      sparse = where(mask, logits, -inf)
    DMA out gate/sparse/idx tiles
"""

import sys

for _p in ("/opt/trn_rl_repo", "/root/.axon_site"):
    if _p not in sys.path:
        sys.path.insert(0, _p)

from contextlib import ExitStack

import numpy as np

import concourse.bacc as bacc
import concourse.bass as bass
import concourse.mybir as mybir
import concourse.tile as tile
from concourse.bass_utils import run_bass_kernel_spmd

B, S, H, E, TOPK = 4, 4096, 2048, 8, 2
N_CORES = 8
TOKENS = B * S
TOK_PER_CORE = TOKENS // N_CORES  # 2048
P = 128                           # partition tile (tokens per tile)
N_TILES = TOK_PER_CORE // P       # 16
HC = H // P                       # 16 h-chunks
F32 = mybir.dt.float32
NEG_INF = float("-inf")

_cache = {}
_EYE = np.eye(P, dtype=np.float32)


def _build():
    nc = bacc.Bacc("TRN2", target_bir_lowering=False, debug=False)

    x_d = nc.dram_tensor("x", [TOK_PER_CORE, H], F32, kind="ExternalInput")
    wt_d = nc.dram_tensor("wt", [P, HC * E], F32, kind="ExternalInput")
    bb_d = nc.dram_tensor("bb", [P, E], F32, kind="ExternalInput")
    id_d = nc.dram_tensor("ident", [P, P], F32, kind="ExternalInput")
    gate_d = nc.dram_tensor("gate", [TOK_PER_CORE, E], F32, kind="ExternalOutput")
    sp_d = nc.dram_tensor("sparse", [TOK_PER_CORE, E], F32, kind="ExternalOutput")
    idx_d = nc.dram_tensor("idx", [TOK_PER_CORE, TOPK], mybir.dt.int32,
                           kind="ExternalOutput")

    with tile.TileContext(nc) as tc:
        with ExitStack() as ctx:
            consts = ctx.enter_context(tc.tile_pool(name="consts", bufs=1))
            xpool = ctx.enter_context(tc.tile_pool(name="xin", bufs=3))
            xtpool = ctx.enter_context(tc.tile_pool(name="xT", bufs=8))
            trps = ctx.enter_context(
                tc.tile_pool(name="trps", bufs=5, space="PSUM"))
            lgps = ctx.enter_context(
                tc.tile_pool(name="lgps", bufs=2, space="PSUM"))
            spool = ctx.enter_context(tc.tile_pool(name="small", bufs=6))

            wt_sb = consts.tile([P, HC * E], F32)
            nc.sync.dma_start(wt_sb[:], wt_d[:, :])
            bb_sb = consts.tile([P, E], F32)
            nc.sync.dma_start(bb_sb[:], bb_d[:, :])
            ident = consts.tile([P, P], F32)
            nc.sync.dma_start(ident[:], id_d[:, :])

            for i in range(N_TILES):
                xt = xpool.tile([P, H], F32)
                nc.sync.dma_start(xt[:], x_d[i * P:(i + 1) * P, :])

                xTg = []
                # 4 transposes per PSUM bank tile, then one wide copy out
                for g in range(4):
                    ps = trps.tile([P, 512], F32, tag="trp")
                    for j in range(4):
                        c = g * 4 + j
                        nc.tensor.transpose(
                            ps[:, j * P:(j + 1) * P],
                            xt[:, c * P:(c + 1) * P], ident[:])
                    xT = xtpool.tile([P, 512], F32, tag="xT")
                    if g % 2 == 0:
                        nc.vector.tensor_copy(xT[:], ps[:])
                    else:
                        nc.scalar.copy(xT[:], ps[:])
                    xTg.append(xT)

                lg = lgps.tile([P, E], F32, tag="lg")
                for c in range(HC):
                    nc.tensor.matmul(
                        lg[:],
                        xTg[c // 4][:, (c % 4) * P:(c % 4 + 1) * P],
                        wt_sb[:, c * E:(c + 1) * E],
                        start=(c == 0),
                        stop=(c == HC - 1),
                    )

                gate = spool.tile([P, E], F32, tag="gate")
                nc.vector.tensor_add(gate[:], lg[:], bb_sb[:])

                mx8 = spool.tile([P, 8], F32, tag="mx8")
                nc.vector.max(out=mx8[:], in_=gate[:])
                ix8 = spool.tile([P, 8], mybir.dt.uint32, tag="ix8")
                nc.vector.max_index(out=ix8[:], in_max=mx8[:], in_values=gate[:])
                ix2 = spool.tile([P, TOPK], mybir.dt.int32, tag="ix2")
                nc.vector.tensor_copy(ix2[:], ix8[:, 0:TOPK])

                keep = spool.tile([P, E], mybir.dt.uint32, tag="keep")
                nc.vector.tensor_scalar(
                    out=keep[:], in0=gate[:], scalar1=mx8[:, 1:2], scalar2=None,
                    op0=mybir.AluOpType.is_ge)
                sp = spool.tile([P, E], F32, tag="sp")
                nc.vector.memset(sp[:], NEG_INF)
                nc.vector.copy_predicated(sp[:], keep[:], gate[:])

                nc.scalar.dma_start(gate_d[i * P:(i + 1) * P, :], gate[:])
                nc.scalar.dma_start(sp_d[i * P:(i + 1) * P, :], sp[:])
                nc.scalar.dma_start(idx_d[i * P:(i + 1) * P, :], ix2[:])

    nc.compile()
    return nc


def _get_nc():
    if "nc" not in _cache:
        _cache["nc"] = _build()
    return _cache["nc"]


def _make_in_maps(x, W, b):
    x = np.ascontiguousarray(np.asarray(x, dtype=np.float32)).reshape(TOKENS, H)
    W = np.asarray(W, dtype=np.float32)
    b = np.asarray(b, dtype=np.float32)

    # Wt packed: [p, c*8+e] = W[e, c*128+p]
    wt = np.ascontiguousarray(
        W.T.reshape(HC, P, E).transpose(1, 0, 2).reshape(P, HC * E))
    bb = np.ascontiguousarray(np.broadcast_to(b, (P, E)))

    return [
        {"x": x[i * TOK_PER_CORE:(i + 1) * TOK_PER_CORE], "wt": wt, "bb": bb,
         "ident": _EYE}
        for i in range(N_CORES)
    ]


def kernel(x: np.ndarray, W: np.ndarray, b: np.ndarray):
    in_maps = _make_in_maps(x, W, b)
    nc = _get_nc()
    res = run_bass_kernel_spmd(nc, in_maps, list(range(N_CORES)))

    gate = np.concatenate([r["gate"] for r in res.results], axis=0)
    sparse = np.concatenate([r["sparse"] for r in res.results], axis=0)
    idx = np.concatenate([r["idx"] for r in res.results], axis=0)

    return (
        sparse.reshape(B, S, E),
        idx.reshape(B, S, TOPK).astype(np.int32),
        gate.reshape(TOKENS, E),
    )


if __name__ == "__main__":
    rng = np.random.default_rng(0)
    x = rng.standard_normal((B, S, H), dtype=np.float32)
    W = (rng.standard_normal((E, H), dtype=np.float32) / np.sqrt(H)).astype(
        np.float32)
    b = np.zeros((E,), dtype=np.float32)
    sp, ix, gl = kernel(x=x, W=W, b=b)
    print("shapes:", sp.shape, ix.shape, gl.shape, sp.dtype, ix.dtype, gl.dtype)
